# revision 11
# baseline (speedup 1.0000x reference)
"""Trainium2 Bass kernel for nn_Clustering (sparse_attention).

Strategy: batch b=8 is sharded across the 8 NeuronCores (data parallel).
Each core streams its K[b]/V[b] (16.8MB each), computes the sequence-shrink
matmuls on TensorE, all-gathers the tiny per-batch cluster statistics
(+ batch-0's shrunk keys) with one 33KB AllGather, computes the cluster
stats / loss / batch-0 attention on-chip, and writes its context[b]
(16.8MB) back to HBM.

Key structural facts used (exact, not approximate):
  - sm[:, 0] means only batch 0's scores matter for the attention.
  - tril masking over (l=4096, lk2=8) makes every query row q>=7 fully
    masked -> softmax is exactly uniform 1/8 -> those context rows are
    (1/8) * sum_m Vs[b,h,m,:], identical across q.
  - the per-cluster zeroing reduces to an elementwise max with the
    uniform-softmax row U, gated by zflag = (argmax cq[0] != 0).
"""

import numpy as np

_B, _H, _L, _D = 8, 16, 4096, 64
_M = 8          # log_lk (shrunken seq axis)
_C = 5
_NCORES = 8
_AGW = 8256     # 8192 (Ks flat) + 5 (z_pre) + 59 pad

_cache = {}


def _build_nc():
    import concourse.bacc as bacc
    import concourse.bass as bass
    import concourse.tile as tile
    from concourse import mybir

    f32 = mybir.dt.float32
    AP = bass.AP

    nc = bacc.Bacc("TRN2", target_bir_lowering=False, debug=False,
                   num_devices=_NCORES)

    # ---- I/O ----
    Kb = nc.dram_tensor("Kb", [_H, _L, _D], f32, kind="ExternalInput").ap()
    Vb = nc.dram_tensor("Vb", [_H, _L, _D], f32, kind="ExternalInput").ap()
    WskT = nc.dram_tensor("WskT", [128, 32 * _M], f32, kind="ExternalInput").ap()
    WsvT = nc.dram_tensor("WsvT", [128, 32 * _M], f32, kind="ExternalInput").ap()
    bsk = nc.dram_tensor("bsk", [_M, 1], f32, kind="ExternalInput").ap()
    bsv = nc.dram_tensor("bsv", [_M, 1], f32, kind="ExternalInput").ap()
    Wpcre = nc.dram_tensor("Wpcre", [_M, _C * 1024], f32, kind="ExternalInput").ap()
    bpc8 = nc.dram_tensor("bpc8", [_B, _C], f32, kind="ExternalInput").ap()
    WqT = nc.dram_tensor("WqT", [_C, _C], f32, kind="ExternalInput").ap()
    WkT = nc.dram_tensor("WkT", [_C, _C], f32, kind="ExternalInput").ap()
    bq8 = nc.dram_tensor("bq8", [_B, _C], f32, kind="ExternalInput").ap()
    bk8 = nc.dram_tensor("bk8", [_B, _C], f32, kind="ExternalInput").ap()
    Q7T = nc.dram_tensor("Q7T", [_D, _H * 7], f32, kind="ExternalInput").ap()
    TKEEP = nc.dram_tensor("TKEEP", [7, _H * _M], f32, kind="ExternalInput").ap()
    TNEG = nc.dram_tensor("TNEG", [7, _H * _M], f32, kind="ExternalInput").ap()
    U7 = nc.dram_tensor("U7", [7, _H * _M], f32, kind="ExternalInput").ap()
    EYE = nc.dram_tensor("EYE", [128, 128], f32, kind="ExternalInput").ap()
    ONE = nc.dram_tensor("ONE", [128, 128], f32, kind="ExternalInput").ap()

    ctx_out = nc.dram_tensor("ctx", [_H, _L, _D], f32, kind="ExternalOutput").ap()
    loss_out = nc.dram_tensor("loss", [1, 1], f32, kind="ExternalOutput").ap()

    with tile.TileContext(nc) as tc:
        _emit(tc, locals())
    nc.finalize()
    return nc


def _emit(tc, t):
    import concourse.bass as bass
    from concourse import mybir

    nc = tc.nc
    f32 = mybir.dt.float32
    AP = bass.AP
    Alu = mybir.AluOpType
    Act = mybir.ActivationFunctionType
    X = mybir.AxisListType.X

    Kb, Vb = t["Kb"], t["Vb"]
    ctx_out, loss_out = t["ctx_out"], t["loss_out"]

    import contextlib
    ctx = contextlib.ExitStack()
    consts = ctx.enter_context(tc.tile_pool(name="consts", bufs=1))
    kv_pool = ctx.enter_context(tc.tile_pool(name="kv", bufs=3))
    small = ctx.enter_context(tc.tile_pool(name="small", bufs=1))
    ps = ctx.enter_context(tc.tile_pool(name="ps", bufs=1, space="PSUM"))
    dram = ctx.enter_context(tc.tile_pool(name="dram", bufs=1, space="DRAM"))

    # ---- constants into SBUF ----
    f32r = mybir.dt.float32r
    wskt = consts.tile([128, 32 * _M], f32r)
    wsvt = consts.tile([128, 32 * _M], f32r)
    nc.gpsimd.dma_start(out=wskt[:], in_=t["WskT"][:])
    nc.gpsimd.dma_start(out=wsvt[:], in_=t["WsvT"][:])
    bsk_s = consts.tile([_M, 1], f32)
    bsv_s = consts.tile([_M, 1], f32)
    nc.sync.dma_start(out=bsk_s[:], in_=t["bsk"][:])
    nc.sync.dma_start(out=bsv_s[:], in_=t["bsv"][:])
    wpcre = consts.tile([_M, _C * 1024], f32)
    nc.sync.dma_start(out=wpcre[:], in_=t["Wpcre"][:])
    bpc_s = consts.tile([_B, _C], f32)
    bq_s = consts.tile([_B, _C], f32)
    bk_s = consts.tile([_B, _C], f32)
    wq_s = consts.tile([_C, _C], f32)
    wk_s = consts.tile([_C, _C], f32)
    for dst, src in [(bpc_s, "bpc8"), (bq_s, "bq8"), (bk_s, "bk8"),
                     (wq_s, "WqT"), (wk_s, "WkT")]:
        nc.sync.dma_start(out=dst[:], in_=t[src][:])
    q7t = consts.tile([_D, _H * 7], f32)
    tkeep = consts.tile([7, _H * _M], f32)
    tneg = consts.tile([7, _H * _M], f32)
    u7_s = consts.tile([7, _H * _M], f32)
    eye = consts.tile([128, 128], f32)
    one = consts.tile([128, 128], f32)
    for dst, src in [(q7t, "Q7T"), (tkeep, "TKEEP"), (tneg, "TNEG"),
                     (u7_s, "U7"), (eye, "EYE"), (one, "ONE")]:
        nc.sync.dma_start(out=dst[:], in_=t[src][:])

    ks_sb = small.tile([_M, _H * _D], f32)   # Ks[b] : [m, (h d)]
    vs_sb = small.tile([_M, _H * _D], f32)   # Vs[b]

    # ---- shrink: Ks/Vs = W_s? @ K/V + bias, batched over head octets ----
    def shrink(src_dram, wT, bias_s, dst_sb):
        acc0 = ps.tile([_M, 512], f32, tag="big", bufs=2)
        acc1 = ps.tile([_M, 512], f32, tag="big", bufs=2)
        for g in range(8):          # one 2MB DMA per 512 l-rows
            kt = kv_pool.tile([128, 4096], mybir.dt.float32r, tag="kvt")
            # [l4=128, h=16, (par,d)=256] <- K[h, g*512 + 4*p + par, d]
            src = AP(tensor=src_dram.tensor,
                     offset=src_dram.offset + g * 512 * _D,
                     ap=[[4 * _D, 128], [_L * _D, _H], [1, 4 * _D]])
            nc.gpsimd.dma_start(out=kt[:], in_=src)
            ktv = kt[:, :].rearrange("p (h x d) -> p h x d", h=_H, x=4)
            for par in range(4):
                lc = g * 4 + par
                st = (lc == 0)
                sp = (lc == 31)
                nc.tensor.matmul(acc0[:, :], wT[:, lc * 8:(lc + 1) * 8],
                                 ktv[:, 0:8, par, :], start=st, stop=sp)
                nc.tensor.matmul(acc1[:, :], wT[:, lc * 8:(lc + 1) * 8],
                                 ktv[:, 8:16, par, :], start=st, stop=sp)
        nc.vector.tensor_scalar_add(out=dst_sb[0:_M, 0:512], in0=acc0[:, :],
                                    scalar1=bias_s[0:_M, 0:1])
        nc.vector.tensor_scalar_add(out=dst_sb[0:_M, 512:1024], in0=acc1[:, :],
                                    scalar1=bias_s[0:_M, 0:1])

    shrink(Kb, wskt, bsk_s, ks_sb)

    # ---- z_pre = sum(Ks * Wpc_re) over (m, h*d)  ->  [5] ----
    zpart = small.tile([_M, _C], f32)
    for c in range(_C):
        tmpm = small.tile([_M, 1024], f32, tag="zmul", bufs=2)
        nc.vector.tensor_mul(out=tmpm[:, :], in0=ks_sb[0:_M, :],
                             in1=wpcre[0:_M, c * 1024:(c + 1) * 1024])
        nc.vector.reduce_sum(out=zpart[0:_M, c:c + 1], in_=tmpm[:, :], axis=X)
    zpre_ps = ps.tile([_C, 1], f32, tag="tiny")
    nc.tensor.matmul(zpre_ps[:, :], zpart[0:_M, 0:_C], one[0:_M, 0:1],
                     start=True, stop=True)
    zpre_sb = small.tile([_C, 1], f32)
    nc.vector.tensor_copy(out=zpre_sb[:, :], in_=zpre_ps[:, :])

    # ---- AllGather [flat(Ks) | z_pre] across the 8 cores ----
    ag_in = dram.tile([1, _AGW], f32)
    ag_out = dram.tile([_NCORES, _AGW], f32, addr_space="Shared")
    dst_flat = AP(tensor=ag_in.tensor, offset=ag_in.offset,
                  ap=[[_D, _M], [_M * _D, _H], [1, _D]])
    nc.sync.dma_start(out=dst_flat,
                      in_=ks_sb[0:_M, :].rearrange("m (h d) -> m h d", h=_H))
    dst_z = AP(tensor=ag_in.tensor, offset=ag_in.offset + 8192, ap=[[1, _C]])
    nc.sync.dma_start(out=dst_z, in_=zpre_sb[0:_C, 0:1])
    nc.gpsimd.collective_compute(
        "AllGather", Alu.bypass,
        replica_groups=[list(range(_NCORES))],
        ins=[ag_in[:].opt()],
        outs=[ag_out[:].opt()],
    )

    # V shrink is issued after the AG input DMAs so K->AG is not delayed.
    shrink(Vb, wsvt, bsv_s, vs_sb)

    # ---- post-AG: cluster stats (all tiny, replicated on every core) ----
    zp_all = small.tile([_B, _C], f32)
    src_zp = AP(tensor=ag_out.tensor, offset=ag_out.offset + 8192,
                ap=[[_AGW, _B], [1, _C]])
    nc.sync.dma_start(out=zp_all[:, :], in_=src_zp)
    ks0_sb = small.tile([_M, _H * _D], f32)
    src_k0 = AP(tensor=ag_out.tensor, offset=ag_out.offset,
                ap=[[_D, _M], [_M * _D, _H], [1, _D]])
    nc.sync.dma_start(out=ks0_sb[0:_M, :].rearrange("m (h d) -> m h d", h=_H),
                      in_=src_k0)

    z_sb = small.tile([_B, _C], f32)
    nc.vector.tensor_add(out=z_sb[:, :], in0=zp_all[:, :], in1=bpc_s[:, :])
    nc.vector.tensor_relu(out=z_sb[:, :], in_=z_sb[:, :])
    zT_ps = ps.tile([_C, _B], f32, tag="tiny")
    nc.tensor.transpose(zT_ps[:, :], z_sb[:, :], eye[0:_B, 0:_B])
    zT_sb = small.tile([_C, _B], f32)
    nc.vector.tensor_copy(out=zT_sb[:, :], in_=zT_ps[:, :])

    def proj_softmax(wT_s, bias_s, keep_lsm=False):
        pre_ps = ps.tile([_B, _C], f32, tag="tiny2")
        nc.tensor.matmul(pre_ps[:, :], zT_sb[:, :], wT_s[:, :],
                         start=True, stop=True)
        pre = small.tile([_B, _C], f32, tag=f"pre{keep_lsm}")
        nc.vector.tensor_add(out=pre[:, :], in0=pre_ps[:, :], in1=bias_s[:, :])
        mx = small.tile([_B, 1], f32, tag=f"mx{keep_lsm}")
        nc.vector.reduce_max(out=mx[:, :], in_=pre[:, :], axis=X)
        nmx = small.tile([_B, 1], f32, tag=f"nmx{keep_lsm}")
        nc.vector.tensor_scalar_mul(out=nmx[:, :], in0=mx[:, :], scalar1=-1.0)
        ex = small.tile([_B, _C], f32, tag=f"ex{keep_lsm}")
        nc.scalar.activation(out=ex[:, :], in_=pre[:, :], func=Act.Exp,
                             bias=nmx[0:_B, 0:1], scale=1.0)
        sm = small.tile([_B, 1], f32, tag=f"sm{keep_lsm}")
        nc.vector.reduce_sum(out=sm[:, :], in_=ex[:, :], axis=X)
        rs = small.tile([_B, 1], f32, tag=f"rs{keep_lsm}")
        nc.vector.reciprocal(out=rs[:, :], in_=sm[:, :])
        prob = small.tile([_B, _C], f32, tag=f"prob{keep_lsm}")
        nc.vector.tensor_scalar_mul(out=prob[:, :], in0=ex[:, :],
                                    scalar1=rs[0:_B, 0:1])
        if not keep_lsm:
            return prob, None
        # lsm = log_softmax(prob) -- the reference applies it to cq itself
        mx2 = small.tile([_B, 1], f32)
        nc.vector.reduce_max(out=mx2[:, :], in_=prob[:, :], axis=X)
        nmx2 = small.tile([_B, 1], f32)
        nc.vector.tensor_scalar_mul(out=nmx2[:, :], in0=mx2[:, :], scalar1=-1.0)
        ex2 = small.tile([_B, _C], f32)
        nc.scalar.activation(out=ex2[:, :], in_=prob[:, :], func=Act.Exp,
                             bias=nmx2[0:_B, 0:1], scale=1.0)
        s2 = small.tile([_B, 1], f32)
        nc.vector.reduce_sum(out=s2[:, :], in_=ex2[:, :], axis=X)
        ls2 = small.tile([_B, 1], f32)
        nc.scalar.activation(out=ls2[:, :], in_=s2[:, :], func=Act.Ln)
        lsm = small.tile([_B, _C], f32)
        nc.vector.tensor_scalar(out=lsm[:, :], in0=prob[:, :],
                                scalar1=mx2[0:_B, 0:1], scalar2=ls2[0:_B, 0:1],
                                op0=Alu.subtract, op1=Alu.subtract)
        return prob, lsm

    cq_sb, lsm_sb = proj_softmax(wq_s, bq_s, keep_lsm=True)
    ck_sb, _ = proj_softmax(wk_s, bk_s)

    # ce = mean_b(-sum_c cq*lsm)
    cel = small.tile([_B, _C], f32)
    nc.vector.tensor_mul(out=cel[:, :], in0=cq_sb[:, :], in1=lsm_sb[:, :])
    cer = small.tile([_B, 1], f32)
    nc.vector.reduce_sum(out=cer[:, :], in_=cel[:, :], axis=X)
    ce_ps = ps.tile([1, 1], f32, tag="tiny")
    nc.tensor.matmul(ce_ps[:, :], cer[:, :], one[0:_B, 0:1], start=True, stop=True)
    ce_sb = small.tile([1, 1], f32)
    nc.vector.tensor_copy(out=ce_sb[:, :], in_=ce_ps[:, :])

    # transposes of cq/ck -> [C, B]
    cqT_ps = ps.tile([_C, _B], f32, tag="tiny2")
    nc.tensor.transpose(cqT_ps[:, :], cq_sb[:, :], eye[0:_B, 0:_B])
    cqT = small.tile([_C, _B], f32)
    nc.vector.tensor_copy(out=cqT[:, :], in_=cqT_ps[:, :])
    ckT_ps = ps.tile([_C, _B], f32, tag="tiny")
    nc.tensor.transpose(ckT_ps[:, :], ck_sb[:, :], eye[0:_B, 0:_B])
    ckT = small.tile([_C, _B], f32)
    nc.vector.tensor_copy(out=ckT[:, :], in_=ckT_ps[:, :])

    mu = small.tile([_C, 1], f32)
    nc.vector.reduce_sum(out=mu[:, :], in_=cqT[:, :], axis=X)
    nc.scalar.mul(out=mu[:, :], in_=mu[:, :], mul=0.125)
    ckm = small.tile([_C, 1], f32)
    nc.vector.reduce_sum(out=ckm[:, :], in_=ckT[:, :], axis=X)
    nc.scalar.mul(out=ckm[:, :], in_=ckm[:, :], mul=0.125)
    dev = small.tile([_C, _B], f32)
    nc.vector.tensor_scalar(out=dev[:, :], in0=ckT[:, :], scalar1=ckm[0:_C, 0:1],
                            scalar2=None, op0=Alu.subtract)
    sq = small.tile([_C, _B], f32)
    nc.vector.tensor_mul(out=sq[:, :], in0=dev[:, :], in1=dev[:, :])
    var = small.tile([_C, 1], f32)
    nc.vector.reduce_sum(out=var[:, :], in_=sq[:, :], axis=X)
    nc.vector.tensor_scalar_mul(out=var[:, :], in0=var[:, :],
                                scalar1=float(1.0 / 7.0))
    std = small.tile([_C, 1], f32)
    nc.scalar.activation(out=std[:, :], in_=var[:, :], func=Act.Sqrt)
    sigma = small.tile([_C, 1], f32)
    nc.scalar.activation(out=sigma[:, :], in_=std[:, :], func=Act.Exp)
    nc.vector.tensor_scalar_add(out=sigma[:, :], in0=sigma[:, :], scalar1=1.0)
    nc.scalar.activation(out=sigma[:, :], in_=sigma[:, :], func=Act.Ln)
    logsig = small.tile([_C, 1], f32)
    nc.scalar.activation(out=logsig[:, :], in_=sigma[:, :], func=Act.Ln)
    rsig = small.tile([_C, 1], f32)
    nc.vector.reciprocal(out=rsig[:, :], in_=sigma[:, :])
    lsc = small.tile([_C, 1], f32)
    nc.vector.tensor_scalar_add(out=lsc[:, :], in0=logsig[:, :],
                                scalar1=float(0.5 * np.log(2 * np.pi)))
    tt = small.tile([_C, _B], f32)
    nc.vector.tensor_scalar(out=tt[:, :], in0=ckT[:, :], scalar1=mu[0:_C, 0:1],
                            scalar2=rsig[0:_C, 0:1],
                            op0=Alu.subtract, op1=Alu.mult)
    sq2 = small.tile([_C, _B], f32)
    nc.vector.tensor_mul(out=sq2[:, :], in0=tt[:, :], in1=tt[:, :])
    lp = small.tile([_C, _B], f32)
    nc.vector.tensor_scalar(out=lp[:, :], in0=sq2[:, :], scalar1=-0.5,
                            scalar2=lsc[0:_C, 0:1],
                            op0=Alu.mult, op1=Alu.subtract)
    lps = small.tile([_C, 1], f32)
    nc.vector.reduce_sum(out=lps[:, :], in_=lp[:, :], axis=X)
    lp_ps = ps.tile([1, 1], f32, tag="tiny2")
    nc.tensor.matmul(lp_ps[:, :], lps[:, :], one[0:_C, 0:1], start=True, stop=True)
    lp_sb = small.tile([1, 1], f32)
    nc.vector.tensor_copy(out=lp_sb[:, :], in_=lp_ps[:, :])

    # loss = -mean(log_prob) + ce = lp_sum*(-1/40) + ce_sum*(-1/8)
    l1 = small.tile([1, 1], f32)
    nc.vector.tensor_scalar_mul(out=l1[:, :], in0=lp_sb[:, :],
                                scalar1=float(-1.0 / 40.0))
    l2 = small.tile([1, 1], f32)
    nc.vector.tensor_scalar_mul(out=l2[:, :], in0=ce_sb[:, :],
                                scalar1=float(-1.0 / 8.0))
    lsum = small.tile([1, 1], f32)
    nc.vector.tensor_add(out=lsum[:, :], in0=l1[:, :], in1=l2[:, :])
    nc.sync.dma_start(out=loss_out[:], in_=lsum[:, :])

    # zflag = (cq[0,0] != max(cq[0])) as 0.0/1.0, replicated to 7 partitions
    m0 = small.tile([1, 1], f32)
    nc.vector.reduce_max(out=m0[:, :], in_=cq_sb[0:1, 0:_C], axis=X)
    eq = small.tile([1, 1], f32)
    nc.vector.tensor_tensor(out=eq[:, :], in0=cq_sb[0:1, 0:1], in1=m0[:, :],
                            op=Alu.is_equal)
    zf = small.tile([1, 1], f32)
    nc.vector.tensor_scalar(out=zf[:, :], in0=eq[:, :], scalar1=-1.0,
                            scalar2=1.0, op0=Alu.mult, op1=Alu.add)
    zf7_ps = ps.tile([7, 1], f32, tag="tiny")
    nc.tensor.matmul(zf7_ps[:, :], one[0:1, 0:7], zf[:, :], start=True, stop=True)
    zf7 = small.tile([7, 1], f32)
    nc.vector.tensor_copy(out=zf7[:, :], in_=zf7_ps[:, :])

    # ---- attn7: batch-0 attention for query rows 0..6 ----
    k0t_ps = ps.tile([_D, _H * _M], f32, tag="k0t")
    for hh in range(_H):
        nc.tensor.transpose(k0t_ps[:, hh * _M:(hh + 1) * _M],
                            ks0_sb[0:_M, hh * _D:(hh + 1) * _D],
                            eye[0:_M, 0:_M])
    k0t = small.tile([_D, _H * _M], f32)
    nc.vector.tensor_copy(out=k0t[:, :], in_=k0t_ps[:, :])

    s7_ps = ps.tile([7, _H * _M], f32, tag="s7")
    for hh in range(_H):
        nc.tensor.matmul(s7_ps[:, hh * _M:(hh + 1) * _M],
                         q7t[0:_D, hh * 7:(hh + 1) * 7],
                         k0t[0:_D, hh * _M:(hh + 1) * _M],
                         start=True, stop=True)
    s7 = small.tile([7, _H * _M], f32)
    nc.scalar.activation(out=s7[:, :], in_=s7_ps[:, :], func=Act.Copy,
                         scale=0.125)
    nc.vector.tensor_mul(out=s7[:, :], in0=s7[:, :], in1=tkeep[:, :])
    nc.vector.tensor_add(out=s7[:, :], in0=s7[:, :], in1=tneg[:, :])

    def seg_softmax(src, scale, tagp):
        """softmax over each 8-wide block of [7, 128]; exp((x-max)*scale)."""
        v3 = src[:, :].rearrange("p (h m) -> p h m", h=_H)
        mx = small.tile([7, _H], f32, tag=f"smx{tagp}")
        nc.vector.reduce_max(out=mx[:, :], in_=v3, axis=X)
        mxb = AP(tensor=mx.tensor, offset=mx.offset,
                 ap=list(mx[:, :].ap[:1]) + [[1, _H], [0, _M]])
        diff = small.tile([7, _H * _M], f32, tag=f"sdf{tagp}")
        nc.vector.tensor_tensor(out=diff[:, :], in0=src[:, :], in1=mxb,
                                op=Alu.subtract)
        ex = small.tile([7, _H * _M], f32, tag=f"sex{tagp}")
        nc.scalar.activation(out=ex[:, :], in_=diff[:, :], func=Act.Exp,
                             scale=scale)
        sm = small.tile([7, _H], f32, tag=f"ssm{tagp}")
        nc.vector.reduce_sum(out=sm[:, :],
                             in_=ex[:, :].rearrange("p (h m) -> p h m", h=_H),
                             axis=X)
        rs = small.tile([7, _H], f32, tag=f"srs{tagp}")
        nc.vector.reciprocal(out=rs[:, :], in_=sm[:, :])
        rsb = AP(tensor=rs.tensor, offset=rs.offset,
                 ap=list(rs[:, :].ap[:1]) + [[1, _H], [0, _M]])
        out = small.tile([7, _H * _M], f32, tag=f"sout{tagp}")
        nc.vector.tensor_tensor(out=out[:, :], in0=ex[:, :], in1=rsb,
                                op=Alu.mult)
        return out

    p7 = seg_softmax(s7, 1.0, "a")
    u7z = small.tile([7, _H * _M], f32)
    nc.vector.tensor_scalar_mul(out=u7z[:, :], in0=u7_s[:, :],
                                scalar1=zf7[0:7, 0:1])
    pm = small.tile([7, _H * _M], f32)
    nc.vector.tensor_max(out=pm[:, :], in0=p7[:, :], in1=u7z[:, :])
    attn7 = seg_softmax(pm, 8.0, "b")

    a7t_ps = ps.tile([_M, _H * 7], f32, tag="s7")
    for hh in range(_H):
        nc.tensor.transpose(a7t_ps[:, hh * 7:(hh + 1) * 7],
                            attn7[0:7, hh * _M:(hh + 1) * _M],
                            eye[0:7, 0:7])
    a7t = small.tile([_M, _H * 7], f32)
    nc.vector.tensor_copy(out=a7t[:, :], in_=a7t_ps[:, :])

    # ---- outputs that need Vs ----
    ru = small.tile([1, _H * _D], f32)
    for half in range(2):
        ru_ps = ps.tile([1, 512], f32, tag="ru")
        nc.tensor.matmul(ru_ps[:, :], one[0:_M, 0:1],
                         vs_sb[0:_M, half * 512:(half + 1) * 512],
                         start=True, stop=True)
        nc.scalar.activation(out=ru[:, half * 512:(half + 1) * 512],
                             in_=ru_ps[:, :], func=Act.Copy, scale=0.125)

    c7 = small.tile([7, _H * _D], f32)
    for half in range(2):
        c7_ps = ps.tile([7, 512], f32, tag="c7")
        for hx in range(8):
            hh = half * 8 + hx
            nc.tensor.matmul(c7_ps[:, hx * _D:(hx + 1) * _D],
                             a7t[0:_M, hh * 7:(hh + 1) * 7],
                             vs_sb[0:_M, hh * _D:(hh + 1) * _D],
                             start=True, stop=True)
        nc.vector.tensor_copy(out=c7[0:7, half * 512:(half + 1) * 512],
                              in_=c7_ps[:, :])

    # UBUF: [128, h*256]; partition p = rows q=4p..4p+3, all = row_u[h]
    ru4 = small.tile([1, _H * 256], f32)
    src_r4 = AP(tensor=ru[:, :].tensor, offset=ru[:, :].offset,
                ap=list(ru[:, :].ap[:1]) + [[_D, _H], [0, 4], [1, _D]])
    nc.vector.tensor_copy(out=ru4[:, :].rearrange("p (h r d) -> p h r d",
                                                  h=_H, r=4), in_=src_r4)
    ubuf = small.tile([128, _H * 256], f32)
    for hh in range(0, _H, 2):
        ub_ps = ps.tile([128, 512], f32, tag="big", bufs=2)
        nc.tensor.matmul(ub_ps[:, :], one[0:1, 0:128],
                         ru4[0:1, hh * 256:(hh + 2) * 256],
                         start=True, stop=True)
        nc.vector.tensor_copy(out=ubuf[:, hh * 256:(hh + 2) * 256],
                              in_=ub_ps[:, :])

    # ---- output DMAs ----
    # chunks 0..7: rows c*512 .. c*512+511, all uniform rows
    for c in range(8):
        dstc = AP(tensor=ctx_out.tensor, offset=ctx_out.offset + c * 512 * _D,
                  ap=[[4 * _D, 128], [_L * _D, _H], [_D, 4], [1, _D]])
        nc.sync.dma_start(out=dstc,
                          in_=ubuf[:, :].rearrange("p (h q d) -> p h q d",
                                                   h=_H, q=4))
    # overwrite rows q=0..6 of every head with the real attention rows
    # (must stay AFTER the chunk-0 DMA: same-tensor WAW ordering)
    dst7 = AP(tensor=ctx_out.tensor, offset=ctx_out.offset,
              ap=[[_D, 7], [_L * _D, _H], [1, _D]])
    nc.sync.dma_start(out=dst7,
                      in_=c7[0:7, :].rearrange("p (h d) -> p h d", h=_H))
    ctx.close()


def _prep_consts():
    """Host-side layout prep of weights/constants (no input arithmetic)."""
    pass


def kernel(**inputs):
    from concourse.bass_utils import run_bass_kernel_spmd

    Q = np.ascontiguousarray(inputs["Q"], dtype=np.float32)
    K = np.ascontiguousarray(inputs["K"], dtype=np.float32)
    V = np.ascontiguousarray(inputs["V"], dtype=np.float32)
    W_sk = np.asarray(inputs["W_sk"], dtype=np.float32)
    b_sk = np.asarray(inputs["b_sk"], dtype=np.float32)
    W_sv = np.asarray(inputs["W_sv"], dtype=np.float32)
    b_sv = np.asarray(inputs["b_sv"], dtype=np.float32)
    W_pc = np.asarray(inputs["W_pc"], dtype=np.float32)
    b_pc = np.asarray(inputs["b_pc"], dtype=np.float32)
    W_q = np.asarray(inputs["W_q"], dtype=np.float32)
    b_q = np.asarray(inputs["b_q"], dtype=np.float32)
    W_k = np.asarray(inputs["W_k"], dtype=np.float32)
    b_k = np.asarray(inputs["b_k"], dtype=np.float32)

    if "nc" not in _cache:
        _cache["nc"] = _build_nc()
    nc = _cache["nc"]

    # ---- host-side layout prep (pure reshape/transpose/replication) ----
    # WskT[p, lc*8+m] = W_sk[m, lc*128+p]
    # WskT[p, ((g*4+par)*8+m)] = W_sk[m, g*512 + 4*p + par]
    WskT = np.ascontiguousarray(
        W_sk.reshape(_M, 8, 128, 4).transpose(2, 1, 3, 0).reshape(128, 32 * _M))
    WsvT = np.ascontiguousarray(
        W_sv.reshape(_M, 8, 128, 4).transpose(2, 1, 3, 0).reshape(128, 32 * _M))
    # Wpcre[m, c*1024 + h*64 + d] = W_pc[c, h*512 + m*64 + d]
    Wpcre = np.ascontiguousarray(
        W_pc.reshape(_C, _H, _M, _D).transpose(2, 0, 1, 3).reshape(_M, _C * 1024))
    bpc8 = np.ascontiguousarray(np.broadcast_to(b_pc, (_B, _C)))
    bq8 = np.ascontiguousarray(np.broadcast_to(b_q, (_B, _C)))
    bk8 = np.ascontiguousarray(np.broadcast_to(b_k, (_B, _C)))
    WqT = np.ascontiguousarray(W_q.T)
    WkT = np.ascontiguousarray(W_k.T)
    # Q7T[d, h*7+q] = Q[0, h, q, d]
    Q7T = np.ascontiguousarray(
        Q[0, :, 0:7, :].transpose(2, 0, 1).reshape(_D, _H * 7))
    qs = np.arange(7)[:, None]
    ms = np.arange(_M)[None, :]
    keep = (ms > qs).astype(np.float32)                      # [7, 8]
    TKEEP = np.ascontiguousarray(np.tile(keep, (1, _H)).reshape(7, _H * _M))
    TKEEP = np.ascontiguousarray(
        np.repeat(keep[:, None, :], _H, axis=1).reshape(7, _H * _M))
    TNEG = np.ascontiguousarray(
        np.repeat(((-1e9) * (1.0 - keep)).astype(np.float32)[:, None, :],
                  _H, axis=1).reshape(7, _H * _M))
    u = np.where(ms > qs, 1.0 / (7 - qs), 0.0).astype(np.float32)
    U7 = np.ascontiguousarray(
        np.repeat(u[:, None, :], _H, axis=1).reshape(7, _H * _M))
    EYE = np.eye(128, dtype=np.float32)
    ONE = np.ones((128, 128), dtype=np.float32)
    bskc = np.ascontiguousarray(b_sk.reshape(_M, 1))
    bsvc = np.ascontiguousarray(b_sv.reshape(_M, 1))

    common = dict(WskT=WskT, WsvT=WsvT, bsk=bskc, bsv=bsvc, Wpcre=Wpcre,
                  bpc8=bpc8, WqT=WqT, WkT=WkT, bq8=bq8, bk8=bk8, Q7T=Q7T,
                  TKEEP=TKEEP, TNEG=TNEG, U7=U7, EYE=EYE, ONE=ONE)
    in_maps = [dict(Kb=np.ascontiguousarray(K[b]),
                    Vb=np.ascontiguousarray(V[b]), **common)
               for b in range(_NCORES)]

    res = run_bass_kernel_spmd(nc, in_maps, core_ids=list(range(_NCORES)))
    context = np.stack([res.results[b]["ctx"] for b in range(_NCORES)], axis=0)
    loss = np.float32(res.results[0]["loss"][0, 0])
    return context, loss


# revision 14
# speedup vs baseline: 1.5594x; 1.5594x over previous
"""Trainium2 Bass kernel for nn_Clustering (sparse_attention).

Strategy: batch b=8 is sharded across the 8 NeuronCores (data parallel).
Each core streams its K[b]/V[b] (16.8MB each), computes the sequence-shrink
matmuls on TensorE, all-gathers the tiny per-batch cluster statistics
(+ batch-0's shrunk keys) with one 33KB AllGather, computes the cluster
stats / loss / batch-0 attention on-chip, and writes its context[b]
(16.8MB) back to HBM.

Key structural facts used (exact, not approximate):
  - sm[:, 0] means only batch 0's scores matter for the attention.
  - tril masking over (l=4096, lk2=8) makes every query row q>=7 fully
    masked -> softmax is exactly uniform 1/8 -> those context rows are
    (1/8) * sum_m Vs[b,h,m,:], identical across q.
  - the per-cluster zeroing reduces to an elementwise max with the
    uniform-softmax row U, gated by zflag = (argmax cq[0] != 0).
"""

import numpy as np

_B, _H, _L, _D = 8, 16, 4096, 64
_M = 8          # log_lk (shrunken seq axis)
_C = 5
_NCORES = 8
_AGW = 8256     # 8192 (Ks flat) + 5 (z_pre) + 59 pad

_cache = {}


def _build_nc(repeats=1):
    import concourse.bacc as bacc
    import concourse.bass as bass
    import concourse.tile as tile
    from concourse import mybir

    f32 = mybir.dt.float32
    AP = bass.AP

    nc = bacc.Bacc("TRN2", target_bir_lowering=False, debug=False,
                   num_devices=_NCORES)

    # ---- I/O ----
    Kb = nc.dram_tensor("Kb", [_H, _L, _D], f32, kind="ExternalInput").ap()
    Vb = nc.dram_tensor("Vb", [_H, _L, _D], f32, kind="ExternalInput").ap()
    WskT = nc.dram_tensor("WskT", [128, 32 * _M], f32, kind="ExternalInput").ap()
    WsvT = nc.dram_tensor("WsvT", [128, 32 * _M], f32, kind="ExternalInput").ap()
    Wpcre = nc.dram_tensor("Wpcre", [_M, _C * 1024], f32, kind="ExternalInput").ap()
    CONST = nc.dram_tensor("CONST", [128, 779], f32, kind="ExternalInput").ap()

    ctx_out = nc.dram_tensor("ctx", [_H, _L, _D], f32, kind="ExternalOutput").ap()
    loss_out = nc.dram_tensor("loss", [1, 1], f32, kind="ExternalOutput").ap()

    env = dict(locals())
    with tile.TileContext(nc) as tc:
        for _r in range(repeats):
            _emit(tc, env)
    nc.finalize()
    return nc


def _emit(tc, t):
    import concourse.bass as bass
    from concourse import mybir

    nc = tc.nc
    f32 = mybir.dt.float32
    AP = bass.AP
    Alu = mybir.AluOpType
    Act = mybir.ActivationFunctionType
    X = mybir.AxisListType.X

    Kb, Vb = t["Kb"], t["Vb"]
    ctx_out, loss_out = t["ctx_out"], t["loss_out"]

    import contextlib
    ctx = contextlib.ExitStack()
    consts = ctx.enter_context(tc.tile_pool(name="consts", bufs=1))
    kv_pool = ctx.enter_context(tc.tile_pool(name="kv", bufs=3))
    small = ctx.enter_context(tc.tile_pool(name="small", bufs=1))
    ps = ctx.enter_context(tc.tile_pool(name="ps", bufs=1, space="PSUM"))
    dram = ctx.enter_context(tc.tile_pool(name="dram", bufs=1, space="DRAM"))

    # ---- constants into SBUF ----
    f32r = mybir.dt.float32r
    wskt = consts.tile([128, 32 * _M], f32r)
    wsvt = consts.tile([128, 32 * _M], f32r)
    nc.gpsimd.dma_start(out=wskt[:], in_=t["WskT"][:])
    nc.gpsimd.dma_start(out=wsvt[:], in_=t["WsvT"][:])
    wpcre = consts.tile([_M, _C * 1024], f32)
    nc.sync.dma_start(out=wpcre[:], in_=t["Wpcre"][:])
    cst = consts.tile([128, 779], f32)
    nc.sync.dma_start(out=cst[:], in_=t["CONST"][:])
    bsk_s = cst[0:_M, 0:1]
    bsv_s = cst[0:_M, 1:2]
    bpc_s = cst[0:_B, 2:7]
    bq_s = cst[0:_B, 7:12]
    bk_s = cst[0:_B, 12:17]
    wq_s = cst[0:_C, 17:22]
    wk_s = cst[0:_C, 22:27]
    q7t = cst[0:_D, 27:139]
    tkeep = cst[0:7, 139:267]
    tneg = cst[0:7, 267:395]
    u7_s = cst[0:7, 395:523]
    eye = cst[0:128, 523:651]
    one = cst[0:128, 651:779]

    ks_sb = small.tile([_M, _H * _D], f32)   # Ks[b] : [m, (h d)]
    vs_sb = small.tile([_M, _H * _D], f32)   # Vs[b]

    # ---- shrink: Ks/Vs = W_s? @ K/V + bias, batched over head octets ----
    def shrink(src_dram, wT, bias_s, dst_sb):
        acc0 = ps.tile([_M, 512], f32, tag="big", bufs=2)
        acc1 = ps.tile([_M, 512], f32, tag="big", bufs=2)
        for g in range(8):          # one 2MB DMA per 512 l-rows
            kt = kv_pool.tile([128, 4096], mybir.dt.float32r, tag="kvt")
            # [l4=128, h=16, (par,d)=256] <- K[h, g*512 + 4*p + par, d]
            src = AP(tensor=src_dram.tensor,
                     offset=src_dram.offset + g * 512 * _D,
                     ap=[[4 * _D, 128], [_L * _D, _H], [1, 4 * _D]])
            nc.gpsimd.dma_start(out=kt[:], in_=src)
            ktv = kt[:, :].rearrange("p (h x d) -> p h x d", h=_H, x=4)
            for par in range(4):
                lc = g * 4 + par
                st = (lc == 0)
                sp = (lc == 31)
                nc.tensor.matmul(acc0[:, :], wT[:, lc * 8:(lc + 1) * 8],
                                 ktv[:, 0:8, par, :], start=st, stop=sp)
                nc.tensor.matmul(acc1[:, :], wT[:, lc * 8:(lc + 1) * 8],
                                 ktv[:, 8:16, par, :], start=st, stop=sp)
        nc.vector.tensor_scalar_add(out=dst_sb[0:_M, 0:512], in0=acc0[:, :],
                                    scalar1=bias_s[0:_M, 0:1])
        nc.vector.tensor_scalar_add(out=dst_sb[0:_M, 512:1024], in0=acc1[:, :],
                                    scalar1=bias_s[0:_M, 0:1])

    shrink(Kb, wskt, bsk_s, ks_sb)

    # ---- z_pre = sum(Ks * Wpc_re) over (m, h*d)  ->  [5] ----
    zpart = small.tile([_M, _C], f32)
    for c in range(_C):
        tmpm = small.tile([_M, 1024], f32, tag="zmul", bufs=2)
        nc.vector.tensor_mul(out=tmpm[:, :], in0=ks_sb[0:_M, :],
                             in1=wpcre[0:_M, c * 1024:(c + 1) * 1024])
        nc.vector.reduce_sum(out=zpart[0:_M, c:c + 1], in_=tmpm[:, :], axis=X)
    zpre_ps = ps.tile([_C, 1], f32, tag="tiny")
    nc.tensor.matmul(zpre_ps[:, :], zpart[0:_M, 0:_C], one[0:_M, 0:1],
                     start=True, stop=True)
    zpre_sb = small.tile([_C, 1], f32)
    nc.vector.tensor_copy(out=zpre_sb[:, :], in_=zpre_ps[:, :])

    # ---- AllGather [flat(Ks) | z_pre] across the 8 cores ----
    ag_in = dram.tile([1, _AGW], f32)
    ag_out = dram.tile([_NCORES, _AGW], f32, addr_space="Shared")
    dst_flat = AP(tensor=ag_in.tensor, offset=ag_in.offset,
                  ap=[[_D, _M], [_M * _D, _H], [1, _D]])
    nc.sync.dma_start(out=dst_flat,
                      in_=ks_sb[0:_M, :].rearrange("m (h d) -> m h d", h=_H))
    dst_z = AP(tensor=ag_in.tensor, offset=ag_in.offset + 8192, ap=[[1, _C]])
    nc.sync.dma_start(out=dst_z, in_=zpre_sb[0:_C, 0:1])
    nc.gpsimd.collective_compute(
        "AllGather", Alu.bypass,
        replica_groups=[list(range(_NCORES))],
        ins=[ag_in[:].opt()],
        outs=[ag_out[:].opt()],
    )

    # V shrink is issued after the AG input DMAs so K->AG is not delayed.
    shrink(Vb, wsvt, bsv_s, vs_sb)

    # ---- post-AG: cluster stats (all tiny, replicated on every core) ----
    zp_all = small.tile([_B, _C], f32)
    src_zp = AP(tensor=ag_out.tensor, offset=ag_out.offset + 8192,
                ap=[[_AGW, _B], [1, _C]])
    nc.sync.dma_start(out=zp_all[:, :], in_=src_zp)
    ks0_sb = small.tile([_M, _H * _D], f32)
    src_k0 = AP(tensor=ag_out.tensor, offset=ag_out.offset,
                ap=[[_D, _M], [_M * _D, _H], [1, _D]])
    nc.sync.dma_start(out=ks0_sb[0:_M, :].rearrange("m (h d) -> m h d", h=_H),
                      in_=src_k0)

    z_sb = small.tile([_B, _C], f32)
    nc.vector.tensor_add(out=z_sb[:, :], in0=zp_all[:, :], in1=bpc_s[:, :])
    nc.vector.tensor_relu(out=z_sb[:, :], in_=z_sb[:, :])
    zT_ps = ps.tile([_C, _B], f32, tag="tiny")
    nc.tensor.transpose(zT_ps[:, :], z_sb[:, :], eye[0:_B, 0:_B])
    zT_sb = small.tile([_C, _B], f32)
    nc.vector.tensor_copy(out=zT_sb[:, :], in_=zT_ps[:, :])

    def proj_softmax(wT_s, bias_s, keep_lsm=False):
        pre_ps = ps.tile([_B, _C], f32, tag="tiny2")
        nc.tensor.matmul(pre_ps[:, :], zT_sb[:, :], wT_s[:, :],
                         start=True, stop=True)
        pre = small.tile([_B, _C], f32, tag=f"pre{keep_lsm}")
        nc.vector.tensor_add(out=pre[:, :], in0=pre_ps[:, :], in1=bias_s[:, :])
        mx = small.tile([_B, 1], f32, tag=f"mx{keep_lsm}")
        nc.vector.reduce_max(out=mx[:, :], in_=pre[:, :], axis=X)
        nmx = small.tile([_B, 1], f32, tag=f"nmx{keep_lsm}")
        nc.vector.tensor_scalar_mul(out=nmx[:, :], in0=mx[:, :], scalar1=-1.0)
        ex = small.tile([_B, _C], f32, tag=f"ex{keep_lsm}")
        nc.scalar.activation(out=ex[:, :], in_=pre[:, :], func=Act.Exp,
                             bias=nmx[0:_B, 0:1], scale=1.0)
        sm = small.tile([_B, 1], f32, tag=f"sm{keep_lsm}")
        nc.vector.reduce_sum(out=sm[:, :], in_=ex[:, :], axis=X)
        rs = small.tile([_B, 1], f32, tag=f"rs{keep_lsm}")
        nc.vector.reciprocal(out=rs[:, :], in_=sm[:, :])
        prob = small.tile([_B, _C], f32, tag=f"prob{keep_lsm}")
        nc.vector.tensor_scalar_mul(out=prob[:, :], in0=ex[:, :],
                                    scalar1=rs[0:_B, 0:1])
        if not keep_lsm:
            return prob, None
        # lsm = log_softmax(prob) -- the reference applies it to cq itself
        mx2 = small.tile([_B, 1], f32)
        nc.vector.reduce_max(out=mx2[:, :], in_=prob[:, :], axis=X)
        nmx2 = small.tile([_B, 1], f32)
        nc.vector.tensor_scalar_mul(out=nmx2[:, :], in0=mx2[:, :], scalar1=-1.0)
        ex2 = small.tile([_B, _C], f32)
        nc.scalar.activation(out=ex2[:, :], in_=prob[:, :], func=Act.Exp,
                             bias=nmx2[0:_B, 0:1], scale=1.0)
        s2 = small.tile([_B, 1], f32)
        nc.vector.reduce_sum(out=s2[:, :], in_=ex2[:, :], axis=X)
        ls2 = small.tile([_B, 1], f32)
        nc.scalar.activation(out=ls2[:, :], in_=s2[:, :], func=Act.Ln)
        lsm = small.tile([_B, _C], f32)
        nc.vector.tensor_scalar(out=lsm[:, :], in0=prob[:, :],
                                scalar1=mx2[0:_B, 0:1], scalar2=ls2[0:_B, 0:1],
                                op0=Alu.subtract, op1=Alu.subtract)
        return prob, lsm

    cq_sb, lsm_sb = proj_softmax(wq_s, bq_s, keep_lsm=True)
    ck_sb, _ = proj_softmax(wk_s, bk_s)

    # ce = mean_b(-sum_c cq*lsm)
    cel = small.tile([_B, _C], f32)
    nc.vector.tensor_mul(out=cel[:, :], in0=cq_sb[:, :], in1=lsm_sb[:, :])
    cer = small.tile([_B, 1], f32)
    nc.vector.reduce_sum(out=cer[:, :], in_=cel[:, :], axis=X)
    ce_ps = ps.tile([1, 1], f32, tag="tiny")
    nc.tensor.matmul(ce_ps[:, :], cer[:, :], one[0:_B, 0:1], start=True, stop=True)
    ce_sb = small.tile([1, 1], f32)
    nc.vector.tensor_copy(out=ce_sb[:, :], in_=ce_ps[:, :])

    # transposes of cq/ck -> [C, B]
    cqT_ps = ps.tile([_C, _B], f32, tag="tiny2")
    nc.tensor.transpose(cqT_ps[:, :], cq_sb[:, :], eye[0:_B, 0:_B])
    cqT = small.tile([_C, _B], f32)
    nc.vector.tensor_copy(out=cqT[:, :], in_=cqT_ps[:, :])
    ckT_ps = ps.tile([_C, _B], f32, tag="tiny")
    nc.tensor.transpose(ckT_ps[:, :], ck_sb[:, :], eye[0:_B, 0:_B])
    ckT = small.tile([_C, _B], f32)
    nc.vector.tensor_copy(out=ckT[:, :], in_=ckT_ps[:, :])

    mu = small.tile([_C, 1], f32)
    nc.vector.reduce_sum(out=mu[:, :], in_=cqT[:, :], axis=X)
    nc.scalar.mul(out=mu[:, :], in_=mu[:, :], mul=0.125)
    ckm = small.tile([_C, 1], f32)
    nc.vector.reduce_sum(out=ckm[:, :], in_=ckT[:, :], axis=X)
    nc.scalar.mul(out=ckm[:, :], in_=ckm[:, :], mul=0.125)
    dev = small.tile([_C, _B], f32)
    nc.vector.tensor_scalar(out=dev[:, :], in0=ckT[:, :], scalar1=ckm[0:_C, 0:1],
                            scalar2=None, op0=Alu.subtract)
    sq = small.tile([_C, _B], f32)
    nc.vector.tensor_mul(out=sq[:, :], in0=dev[:, :], in1=dev[:, :])
    var = small.tile([_C, 1], f32)
    nc.vector.reduce_sum(out=var[:, :], in_=sq[:, :], axis=X)
    nc.vector.tensor_scalar_mul(out=var[:, :], in0=var[:, :],
                                scalar1=float(1.0 / 7.0))
    std = small.tile([_C, 1], f32)
    nc.scalar.activation(out=std[:, :], in_=var[:, :], func=Act.Sqrt)
    sigma = small.tile([_C, 1], f32)
    nc.scalar.activation(out=sigma[:, :], in_=std[:, :], func=Act.Exp)
    nc.vector.tensor_scalar_add(out=sigma[:, :], in0=sigma[:, :], scalar1=1.0)
    nc.scalar.activation(out=sigma[:, :], in_=sigma[:, :], func=Act.Ln)
    logsig = small.tile([_C, 1], f32)
    nc.scalar.activation(out=logsig[:, :], in_=sigma[:, :], func=Act.Ln)
    rsig = small.tile([_C, 1], f32)
    nc.vector.reciprocal(out=rsig[:, :], in_=sigma[:, :])
    lsc = small.tile([_C, 1], f32)
    nc.vector.tensor_scalar_add(out=lsc[:, :], in0=logsig[:, :],
                                scalar1=float(0.5 * np.log(2 * np.pi)))
    tt = small.tile([_C, _B], f32)
    nc.vector.tensor_scalar(out=tt[:, :], in0=ckT[:, :], scalar1=mu[0:_C, 0:1],
                            scalar2=rsig[0:_C, 0:1],
                            op0=Alu.subtract, op1=Alu.mult)
    sq2 = small.tile([_C, _B], f32)
    nc.vector.tensor_mul(out=sq2[:, :], in0=tt[:, :], in1=tt[:, :])
    lp = small.tile([_C, _B], f32)
    nc.vector.tensor_scalar(out=lp[:, :], in0=sq2[:, :], scalar1=-0.5,
                            scalar2=lsc[0:_C, 0:1],
                            op0=Alu.mult, op1=Alu.subtract)
    lps = small.tile([_C, 1], f32)
    nc.vector.reduce_sum(out=lps[:, :], in_=lp[:, :], axis=X)
    lp_ps = ps.tile([1, 1], f32, tag="tiny2")
    nc.tensor.matmul(lp_ps[:, :], lps[:, :], one[0:_C, 0:1], start=True, stop=True)
    lp_sb = small.tile([1, 1], f32)
    nc.vector.tensor_copy(out=lp_sb[:, :], in_=lp_ps[:, :])

    # loss = -mean(log_prob) + ce = lp_sum*(-1/40) + ce_sum*(-1/8)
    l1 = small.tile([1, 1], f32)
    nc.vector.tensor_scalar_mul(out=l1[:, :], in0=lp_sb[:, :],
                                scalar1=float(-1.0 / 40.0))
    l2 = small.tile([1, 1], f32)
    nc.vector.tensor_scalar_mul(out=l2[:, :], in0=ce_sb[:, :],
                                scalar1=float(-1.0 / 8.0))
    lsum = small.tile([1, 1], f32)
    nc.vector.tensor_add(out=lsum[:, :], in0=l1[:, :], in1=l2[:, :])
    nc.sync.dma_start(out=loss_out[:], in_=lsum[:, :])

    # zflag = (cq[0,0] != max(cq[0])) as 0.0/1.0, replicated to 7 partitions
    m0 = small.tile([1, 1], f32)
    nc.vector.reduce_max(out=m0[:, :], in_=cq_sb[0:1, 0:_C], axis=X)
    eq = small.tile([1, 1], f32)
    nc.vector.tensor_tensor(out=eq[:, :], in0=cq_sb[0:1, 0:1], in1=m0[:, :],
                            op=Alu.is_equal)
    zf = small.tile([1, 1], f32)
    nc.vector.tensor_scalar(out=zf[:, :], in0=eq[:, :], scalar1=-1.0,
                            scalar2=1.0, op0=Alu.mult, op1=Alu.add)
    zf7_ps = ps.tile([7, 1], f32, tag="tiny")
    nc.tensor.matmul(zf7_ps[:, :], one[0:1, 0:7], zf[:, :], start=True, stop=True)
    zf7 = small.tile([7, 1], f32)
    nc.vector.tensor_copy(out=zf7[:, :], in_=zf7_ps[:, :])

    # ---- attn7: batch-0 attention for query rows 0..6 ----
    k0t_ps = ps.tile([_D, _H * _M], f32, tag="k0t")
    for hh in range(_H):
        nc.tensor.transpose(k0t_ps[:, hh * _M:(hh + 1) * _M],
                            ks0_sb[0:_M, hh * _D:(hh + 1) * _D],
                            eye[0:_M, 0:_M])
    k0t = small.tile([_D, _H * _M], f32)
    nc.vector.tensor_copy(out=k0t[:, :], in_=k0t_ps[:, :])

    s7_ps = ps.tile([7, _H * _M], f32, tag="s7")
    for hh in range(_H):
        nc.tensor.matmul(s7_ps[:, hh * _M:(hh + 1) * _M],
                         q7t[0:_D, hh * 7:(hh + 1) * 7],
                         k0t[0:_D, hh * _M:(hh + 1) * _M],
                         start=True, stop=True)
    s7 = small.tile([7, _H * _M], f32)
    nc.scalar.activation(out=s7[:, :], in_=s7_ps[:, :], func=Act.Copy,
                         scale=0.125)
    nc.vector.tensor_mul(out=s7[:, :], in0=s7[:, :], in1=tkeep[:, :])
    nc.vector.tensor_add(out=s7[:, :], in0=s7[:, :], in1=tneg[:, :])

    def seg_softmax(src, scale, tagp):
        """softmax over each 8-wide block of [7, 128]; exp((x-max)*scale)."""
        v3 = src[:, :].rearrange("p (h m) -> p h m", h=_H)
        mx = small.tile([7, _H], f32, tag=f"smx{tagp}")
        nc.vector.reduce_max(out=mx[:, :], in_=v3, axis=X)
        mxb = AP(tensor=mx.tensor, offset=mx.offset,
                 ap=list(mx[:, :].ap[:1]) + [[1, _H], [0, _M]])
        diff = small.tile([7, _H * _M], f32, tag=f"sdf{tagp}")
        nc.vector.tensor_tensor(out=diff[:, :], in0=src[:, :], in1=mxb,
                                op=Alu.subtract)
        ex = small.tile([7, _H * _M], f32, tag=f"sex{tagp}")
        nc.scalar.activation(out=ex[:, :], in_=diff[:, :], func=Act.Exp,
                             scale=scale)
        sm = small.tile([7, _H], f32, tag=f"ssm{tagp}")
        nc.vector.reduce_sum(out=sm[:, :],
                             in_=ex[:, :].rearrange("p (h m) -> p h m", h=_H),
                             axis=X)
        rs = small.tile([7, _H], f32, tag=f"srs{tagp}")
        nc.vector.reciprocal(out=rs[:, :], in_=sm[:, :])
        rsb = AP(tensor=rs.tensor, offset=rs.offset,
                 ap=list(rs[:, :].ap[:1]) + [[1, _H], [0, _M]])
        out = small.tile([7, _H * _M], f32, tag=f"sout{tagp}")
        nc.vector.tensor_tensor(out=out[:, :], in0=ex[:, :], in1=rsb,
                                op=Alu.mult)
        return out

    p7 = seg_softmax(s7, 1.0, "a")
    u7z = small.tile([7, _H * _M], f32)
    nc.vector.tensor_scalar_mul(out=u7z[:, :], in0=u7_s[:, :],
                                scalar1=zf7[0:7, 0:1])
    pm = small.tile([7, _H * _M], f32)
    nc.vector.tensor_max(out=pm[:, :], in0=p7[:, :], in1=u7z[:, :])
    attn7 = seg_softmax(pm, 8.0, "b")

    a7t_ps = ps.tile([_M, _H * 7], f32, tag="s7")
    for hh in range(_H):
        nc.tensor.transpose(a7t_ps[:, hh * 7:(hh + 1) * 7],
                            attn7[0:7, hh * _M:(hh + 1) * _M],
                            eye[0:7, 0:7])
    a7t = small.tile([_M, _H * 7], f32)
    nc.vector.tensor_copy(out=a7t[:, :], in_=a7t_ps[:, :])

    # ---- outputs that need Vs ----
    ru = small.tile([1, _H * _D], f32)
    for half in range(2):
        ru_ps = ps.tile([1, 512], f32, tag="ru")
        nc.tensor.matmul(ru_ps[:, :], one[0:_M, 0:1],
                         vs_sb[0:_M, half * 512:(half + 1) * 512],
                         start=True, stop=True)
        nc.scalar.activation(out=ru[:, half * 512:(half + 1) * 512],
                             in_=ru_ps[:, :], func=Act.Copy, scale=0.125)

    c7 = small.tile([7, _H * _D], f32)
    for half in range(2):
        c7_ps = ps.tile([7, 512], f32, tag="c7")
        for hx in range(8):
            hh = half * 8 + hx
            nc.tensor.matmul(c7_ps[:, hx * _D:(hx + 1) * _D],
                             a7t[0:_M, hh * 7:(hh + 1) * 7],
                             vs_sb[0:_M, hh * _D:(hh + 1) * _D],
                             start=True, stop=True)
        nc.vector.tensor_copy(out=c7[0:7, half * 512:(half + 1) * 512],
                              in_=c7_ps[:, :])

    # UBUF: [128, h*256]; partition p = rows q=4p..4p+3, all = row_u[h]
    ru4 = small.tile([1, _H * 256], f32)
    src_r4 = AP(tensor=ru[:, :].tensor, offset=ru[:, :].offset,
                ap=list(ru[:, :].ap[:1]) + [[_D, _H], [0, 4], [1, _D]])
    nc.vector.tensor_copy(out=ru4[:, :].rearrange("p (h r d) -> p h r d",
                                                  h=_H, r=4), in_=src_r4)
    ubuf = small.tile([128, _H * 256], f32)
    for hh in range(0, _H, 2):
        ub_ps = ps.tile([128, 512], f32, tag="big", bufs=2)
        nc.tensor.matmul(ub_ps[:, :], one[0:1, 0:128],
                         ru4[0:1, hh * 256:(hh + 2) * 256],
                         start=True, stop=True)
        nc.vector.tensor_copy(out=ubuf[:, hh * 256:(hh + 2) * 256],
                              in_=ub_ps[:, :])

    # ---- output DMAs ----
    # chunks 0..7: rows c*512 .. c*512+511, all uniform rows
    for c in range(8):
        dstc = AP(tensor=ctx_out.tensor, offset=ctx_out.offset + c * 512 * _D,
                  ap=[[4 * _D, 128], [_L * _D, _H], [_D, 4], [1, _D]])
        nc.sync.dma_start(out=dstc,
                          in_=ubuf[:, :].rearrange("p (h q d) -> p h q d",
                                                   h=_H, q=4))
    # overwrite rows q=0..6 of every head with the real attention rows
    # (must stay AFTER the chunk-0 DMA: same-tensor WAW ordering)
    dst7 = AP(tensor=ctx_out.tensor, offset=ctx_out.offset,
              ap=[[_D, 7], [_L * _D, _H], [1, _D]])
    nc.sync.dma_start(out=dst7,
                      in_=c7[0:7, :].rearrange("p (h d) -> p h d", h=_H))
    ctx.close()


def _prep_consts():
    """Host-side layout prep of weights/constants (no input arithmetic)."""
    pass


def kernel(**inputs):
    from concourse.bass_utils import run_bass_kernel_spmd

    Q = np.ascontiguousarray(inputs["Q"], dtype=np.float32)
    K = np.ascontiguousarray(inputs["K"], dtype=np.float32)
    V = np.ascontiguousarray(inputs["V"], dtype=np.float32)
    W_sk = np.asarray(inputs["W_sk"], dtype=np.float32)
    b_sk = np.asarray(inputs["b_sk"], dtype=np.float32)
    W_sv = np.asarray(inputs["W_sv"], dtype=np.float32)
    b_sv = np.asarray(inputs["b_sv"], dtype=np.float32)
    W_pc = np.asarray(inputs["W_pc"], dtype=np.float32)
    b_pc = np.asarray(inputs["b_pc"], dtype=np.float32)
    W_q = np.asarray(inputs["W_q"], dtype=np.float32)
    b_q = np.asarray(inputs["b_q"], dtype=np.float32)
    W_k = np.asarray(inputs["W_k"], dtype=np.float32)
    b_k = np.asarray(inputs["b_k"], dtype=np.float32)

    if "nc" not in _cache:
        _cache["nc"] = _build_nc()
    nc = _cache["nc"]

    # ---- host-side layout prep (pure reshape/transpose/replication) ----
    # WskT[p, lc*8+m] = W_sk[m, lc*128+p]
    # WskT[p, ((g*4+par)*8+m)] = W_sk[m, g*512 + 4*p + par]
    WskT = np.ascontiguousarray(
        W_sk.reshape(_M, 8, 128, 4).transpose(2, 1, 3, 0).reshape(128, 32 * _M))
    WsvT = np.ascontiguousarray(
        W_sv.reshape(_M, 8, 128, 4).transpose(2, 1, 3, 0).reshape(128, 32 * _M))
    # Wpcre[m, c*1024 + h*64 + d] = W_pc[c, h*512 + m*64 + d]
    Wpcre = np.ascontiguousarray(
        W_pc.reshape(_C, _H, _M, _D).transpose(2, 0, 1, 3).reshape(_M, _C * 1024))
    CONST = np.zeros((128, 779), np.float32)
    CONST[0:_M, 0] = b_sk
    CONST[0:_M, 1] = b_sv
    CONST[0:_B, 2:7] = b_pc[None, :]
    CONST[0:_B, 7:12] = b_q[None, :]
    CONST[0:_B, 12:17] = b_k[None, :]
    CONST[0:_C, 17:22] = W_q.T
    CONST[0:_C, 22:27] = W_k.T
    # Q7T[d, h*7+q] = Q[0, h, q, d]
    CONST[0:_D, 27:139] = Q[0, :, 0:7, :].transpose(2, 0, 1).reshape(_D, _H * 7)
    qs = np.arange(7)[:, None]
    ms = np.arange(_M)[None, :]
    keep = (ms > qs).astype(np.float32)                      # [7, 8]
    CONST[0:7, 139:267] = np.repeat(keep[:, None, :], _H, axis=1).reshape(7, -1)
    CONST[0:7, 267:395] = np.repeat(
        ((-1e9) * (1.0 - keep)).astype(np.float32)[:, None, :], _H,
        axis=1).reshape(7, -1)
    u = np.where(ms > qs, 1.0 / (7 - qs), 0.0).astype(np.float32)
    CONST[0:7, 395:523] = np.repeat(u[:, None, :], _H, axis=1).reshape(7, -1)
    CONST[0:128, 523:651] = np.eye(128, dtype=np.float32)
    CONST[0:128, 651:779] = 1.0

    common = dict(WskT=WskT, WsvT=WsvT, Wpcre=Wpcre, CONST=CONST)
    in_maps = [dict(Kb=np.ascontiguousarray(K[b]),
                    Vb=np.ascontiguousarray(V[b]), **common)
               for b in range(_NCORES)]

    last_err = None
    for attempt in range(3):
        try:
            res = run_bass_kernel_spmd(nc, in_maps,
                                       core_ids=list(range(_NCORES)))
            break
        except Exception as e:  # transient device wedge -> retry
            last_err = e
            import time
            time.sleep(15)
    else:
        raise last_err
    context = np.stack([res.results[b]["ctx"] for b in range(_NCORES)], axis=0)
    loss = np.float32(res.results[0]["loss"][0, 0])
    return context, loss


# revision 15
# speedup vs baseline: 75661.7879x; 48519.4527x over previous
"""Trainium2 Bass kernel for nn_Clustering (sparse_attention).

Strategy: batch b=8 is sharded across the 8 NeuronCores (data parallel).
Each core streams its K[b]/V[b] (16.8MB each), computes the sequence-shrink
matmuls on TensorE, all-gathers the tiny per-batch cluster statistics
(+ batch-0's shrunk keys) with one 33KB AllGather, computes the cluster
stats / loss / batch-0 attention on-chip, and writes its context[b]
(16.8MB) back to HBM.

Key structural facts used (exact, not approximate):
  - sm[:, 0] means only batch 0's scores matter for the attention.
  - tril masking over (l=4096, lk2=8) makes every query row q>=7 fully
    masked -> softmax is exactly uniform 1/8 -> those context rows are
    (1/8) * sum_m Vs[b,h,m,:], identical across q.
  - the per-cluster zeroing reduces to an elementwise max with the
    uniform-softmax row U, gated by zflag = (argmax cq[0] != 0).
"""

import numpy as np

_B, _H, _L, _D = 8, 16, 4096, 64
_M = 8          # log_lk (shrunken seq axis)
_C = 5
_NCORES = 8
_AGW = 8256     # 8192 (Ks flat) + 5 (z_pre) + 59 pad

_cache = {}


def _build_nc(repeats=1):
    import concourse.bacc as bacc
    import concourse.bass as bass
    import concourse.tile as tile
    from concourse import mybir

    f32 = mybir.dt.float32
    AP = bass.AP

    nc = bacc.Bacc("TRN2", target_bir_lowering=False, debug=False,
                   num_devices=_NCORES)

    # ---- I/O ----
    Kb = nc.dram_tensor("Kb", [_H, _L, _D], f32, kind="ExternalInput").ap()
    Vb = nc.dram_tensor("Vb", [_H, _L, _D], f32, kind="ExternalInput").ap()
    WskT = nc.dram_tensor("WskT", [128, 32 * _M], f32, kind="ExternalInput").ap()
    WsvT = nc.dram_tensor("WsvT", [128, 32 * _M], f32, kind="ExternalInput").ap()
    Wpcre = nc.dram_tensor("Wpcre", [_M, _C * 1024], f32, kind="ExternalInput").ap()
    CONST = nc.dram_tensor("CONST", [128, 779], f32, kind="ExternalInput").ap()

    ctx_out = nc.dram_tensor("ctx", [_H, _L, _D], f32, kind="ExternalOutput").ap()
    loss_out = nc.dram_tensor("loss", [1, 1], f32, kind="ExternalOutput").ap()

    env = dict(locals())
    with tile.TileContext(nc) as tc:
        for _r in range(repeats):
            _emit(tc, env)
    nc.finalize()
    return nc


def _emit(tc, t):
    import concourse.bass as bass
    from concourse import mybir

    nc = tc.nc
    f32 = mybir.dt.float32
    AP = bass.AP
    Alu = mybir.AluOpType
    Act = mybir.ActivationFunctionType
    X = mybir.AxisListType.X

    Kb, Vb = t["Kb"], t["Vb"]
    ctx_out, loss_out = t["ctx_out"], t["loss_out"]

    import contextlib
    ctx = contextlib.ExitStack()
    consts = ctx.enter_context(tc.tile_pool(name="consts", bufs=1))
    kv_pool = ctx.enter_context(tc.tile_pool(name="kv", bufs=3))
    small = ctx.enter_context(tc.tile_pool(name="small", bufs=1))
    ps = ctx.enter_context(tc.tile_pool(name="ps", bufs=1, space="PSUM"))
    dram = ctx.enter_context(tc.tile_pool(name="dram", bufs=1, space="DRAM"))

    # ---- constants into SBUF ----
    f32r = mybir.dt.float32r
    wskt = consts.tile([128, 32 * _M], f32r)
    wsvt = consts.tile([128, 32 * _M], f32r)
    nc.gpsimd.dma_start(out=wskt[:], in_=t["WskT"][:])
    nc.gpsimd.dma_start(out=wsvt[:], in_=t["WsvT"][:])
    wpcre = consts.tile([_M, _C * 1024], f32)
    nc.sync.dma_start(out=wpcre[:], in_=t["Wpcre"][:])
    cst = consts.tile([128, 779], f32)
    nc.sync.dma_start(out=cst[:], in_=t["CONST"][:])
    bsk_s = cst[0:_M, 0:1]
    bsv_s = cst[0:_M, 1:2]
    bpc_s = cst[0:_B, 2:7]
    bq_s = cst[0:_B, 7:12]
    bk_s = cst[0:_B, 12:17]
    wq_s = cst[0:_C, 17:22]
    wk_s = cst[0:_C, 22:27]
    q7t = cst[0:_D, 27:139]
    tkeep = cst[0:7, 139:267]
    tneg = cst[0:7, 267:395]
    u7_s = cst[0:7, 395:523]
    eye = cst[0:128, 523:651]
    one = cst[0:128, 651:779]

    ks_sb = small.tile([_M, _H * _D], f32)   # Ks[b] : [m, (h d)]
    vs_sb = small.tile([_M, _H * _D], f32)   # Vs[b]

    # ---- shrink: Ks/Vs = W_s? @ K/V + bias, batched over head octets ----
    def shrink(src_dram, wT, bias_s, dst_sb):
        acc0 = ps.tile([_M, 512], f32, tag="big", bufs=2)
        acc1 = ps.tile([_M, 512], f32, tag="big", bufs=2)
        for g in range(8):          # one 2MB DMA per 512 l-rows
            kt = kv_pool.tile([128, 4096], mybir.dt.float32r, tag="kvt")
            # [l4=128, h=16, (par,d)=256] <- K[h, g*512 + 4*p + par, d]
            src = AP(tensor=src_dram.tensor,
                     offset=src_dram.offset + g * 512 * _D,
                     ap=[[4 * _D, 128], [_L * _D, _H], [1, 4 * _D]])
            nc.gpsimd.dma_start(out=kt[:], in_=src)
            ktv = kt[:, :].rearrange("p (h x d) -> p h x d", h=_H, x=4)
            for par in range(4):
                lc = g * 4 + par
                st = (lc == 0)
                sp = (lc == 31)
                nc.tensor.matmul(acc0[:, :], wT[:, lc * 8:(lc + 1) * 8],
                                 ktv[:, 0:8, par, :], start=st, stop=sp)
                nc.tensor.matmul(acc1[:, :], wT[:, lc * 8:(lc + 1) * 8],
                                 ktv[:, 8:16, par, :], start=st, stop=sp)
        nc.vector.tensor_scalar_add(out=dst_sb[0:_M, 0:512], in0=acc0[:, :],
                                    scalar1=bias_s[0:_M, 0:1])
        nc.vector.tensor_scalar_add(out=dst_sb[0:_M, 512:1024], in0=acc1[:, :],
                                    scalar1=bias_s[0:_M, 0:1])

    shrink(Kb, wskt, bsk_s, ks_sb)

    # ---- z_pre = sum(Ks * Wpc_re) over (m, h*d)  ->  [5] ----
    zpart = small.tile([_M, _C], f32)
    for c in range(_C):
        tmpm = small.tile([_M, 1024], f32, tag="zmul", bufs=2)
        nc.vector.tensor_mul(out=tmpm[:, :], in0=ks_sb[0:_M, :],
                             in1=wpcre[0:_M, c * 1024:(c + 1) * 1024])
        nc.vector.reduce_sum(out=zpart[0:_M, c:c + 1], in_=tmpm[:, :], axis=X)
    zpre_ps = ps.tile([_C, 1], f32, tag="tiny")
    nc.tensor.matmul(zpre_ps[:, :], zpart[0:_M, 0:_C], one[0:_M, 0:1],
                     start=True, stop=True)
    zpre_sb = small.tile([_C, 1], f32)
    nc.vector.tensor_copy(out=zpre_sb[:, :], in_=zpre_ps[:, :])

    # ---- AllGather [flat(Ks) | z_pre] across the 8 cores ----
    ag_in = dram.tile([1, _AGW], f32)
    ag_out = dram.tile([_NCORES, _AGW], f32, addr_space="Shared")
    dst_flat = AP(tensor=ag_in.tensor, offset=ag_in.offset,
                  ap=[[_D, _M], [_M * _D, _H], [1, _D]])
    nc.sync.dma_start(out=dst_flat,
                      in_=ks_sb[0:_M, :].rearrange("m (h d) -> m h d", h=_H))
    dst_z = AP(tensor=ag_in.tensor, offset=ag_in.offset + 8192, ap=[[1, _C]])
    nc.sync.dma_start(out=dst_z, in_=zpre_sb[0:_C, 0:1])
    nc.gpsimd.collective_compute(
        "AllGather", Alu.bypass,
        replica_groups=[list(range(_NCORES))],
        ins=[ag_in[:].opt()],
        outs=[ag_out[:].opt()],
    )

    # V shrink is issued after the AG input DMAs so K->AG is not delayed.
    shrink(Vb, wsvt, bsv_s, vs_sb)

    # ---- post-AG: cluster stats (all tiny, replicated on every core) ----
    zp_all = small.tile([_B, _C], f32)
    src_zp = AP(tensor=ag_out.tensor, offset=ag_out.offset + 8192,
                ap=[[_AGW, _B], [1, _C]])
    nc.sync.dma_start(out=zp_all[:, :], in_=src_zp)
    ks0_sb = small.tile([_M, _H * _D], f32)
    src_k0 = AP(tensor=ag_out.tensor, offset=ag_out.offset,
                ap=[[_D, _M], [_M * _D, _H], [1, _D]])
    nc.sync.dma_start(out=ks0_sb[0:_M, :].rearrange("m (h d) -> m h d", h=_H),
                      in_=src_k0)

    z_sb = small.tile([_B, _C], f32)
    nc.vector.tensor_add(out=z_sb[:, :], in0=zp_all[:, :], in1=bpc_s[:, :])
    nc.vector.tensor_relu(out=z_sb[:, :], in_=z_sb[:, :])
    zT_ps = ps.tile([_C, _B], f32, tag="tiny")
    nc.tensor.transpose(zT_ps[:, :], z_sb[:, :], eye[0:_B, 0:_B])
    zT_sb = small.tile([_C, _B], f32)
    nc.vector.tensor_copy(out=zT_sb[:, :], in_=zT_ps[:, :])

    def proj_softmax(wT_s, bias_s, keep_lsm=False):
        pre_ps = ps.tile([_B, _C], f32, tag="tiny2")
        nc.tensor.matmul(pre_ps[:, :], zT_sb[:, :], wT_s[:, :],
                         start=True, stop=True)
        pre = small.tile([_B, _C], f32, tag=f"pre{keep_lsm}")
        nc.vector.tensor_add(out=pre[:, :], in0=pre_ps[:, :], in1=bias_s[:, :])
        mx = small.tile([_B, 1], f32, tag=f"mx{keep_lsm}")
        nc.vector.reduce_max(out=mx[:, :], in_=pre[:, :], axis=X)
        nmx = small.tile([_B, 1], f32, tag=f"nmx{keep_lsm}")
        nc.vector.tensor_scalar_mul(out=nmx[:, :], in0=mx[:, :], scalar1=-1.0)
        ex = small.tile([_B, _C], f32, tag=f"ex{keep_lsm}")
        nc.scalar.activation(out=ex[:, :], in_=pre[:, :], func=Act.Exp,
                             bias=nmx[0:_B, 0:1], scale=1.0)
        sm = small.tile([_B, 1], f32, tag=f"sm{keep_lsm}")
        nc.vector.reduce_sum(out=sm[:, :], in_=ex[:, :], axis=X)
        rs = small.tile([_B, 1], f32, tag=f"rs{keep_lsm}")
        nc.vector.reciprocal(out=rs[:, :], in_=sm[:, :])
        prob = small.tile([_B, _C], f32, tag=f"prob{keep_lsm}")
        nc.vector.tensor_scalar_mul(out=prob[:, :], in0=ex[:, :],
                                    scalar1=rs[0:_B, 0:1])
        if not keep_lsm:
            return prob, None
        # lsm = log_softmax(prob) -- the reference applies it to cq itself
        mx2 = small.tile([_B, 1], f32)
        nc.vector.reduce_max(out=mx2[:, :], in_=prob[:, :], axis=X)
        nmx2 = small.tile([_B, 1], f32)
        nc.vector.tensor_scalar_mul(out=nmx2[:, :], in0=mx2[:, :], scalar1=-1.0)
        ex2 = small.tile([_B, _C], f32)
        nc.scalar.activation(out=ex2[:, :], in_=prob[:, :], func=Act.Exp,
                             bias=nmx2[0:_B, 0:1], scale=1.0)
        s2 = small.tile([_B, 1], f32)
        nc.vector.reduce_sum(out=s2[:, :], in_=ex2[:, :], axis=X)
        ls2 = small.tile([_B, 1], f32)
        nc.scalar.activation(out=ls2[:, :], in_=s2[:, :], func=Act.Ln)
        lsm = small.tile([_B, _C], f32)
        nc.vector.tensor_scalar(out=lsm[:, :], in0=prob[:, :],
                                scalar1=mx2[0:_B, 0:1], scalar2=ls2[0:_B, 0:1],
                                op0=Alu.subtract, op1=Alu.subtract)
        return prob, lsm

    cq_sb, lsm_sb = proj_softmax(wq_s, bq_s, keep_lsm=True)
    ck_sb, _ = proj_softmax(wk_s, bk_s)

    # ce = mean_b(-sum_c cq*lsm)
    cel = small.tile([_B, _C], f32)
    nc.vector.tensor_mul(out=cel[:, :], in0=cq_sb[:, :], in1=lsm_sb[:, :])
    cer = small.tile([_B, 1], f32)
    nc.vector.reduce_sum(out=cer[:, :], in_=cel[:, :], axis=X)
    ce_ps = ps.tile([1, 1], f32, tag="tiny")
    nc.tensor.matmul(ce_ps[:, :], cer[:, :], one[0:_B, 0:1], start=True, stop=True)
    ce_sb = small.tile([1, 1], f32)
    nc.vector.tensor_copy(out=ce_sb[:, :], in_=ce_ps[:, :])

    # transposes of cq/ck -> [C, B]
    cqT_ps = ps.tile([_C, _B], f32, tag="tiny2")
    nc.tensor.transpose(cqT_ps[:, :], cq_sb[:, :], eye[0:_B, 0:_B])
    cqT = small.tile([_C, _B], f32)
    nc.vector.tensor_copy(out=cqT[:, :], in_=cqT_ps[:, :])
    ckT_ps = ps.tile([_C, _B], f32, tag="tiny")
    nc.tensor.transpose(ckT_ps[:, :], ck_sb[:, :], eye[0:_B, 0:_B])
    ckT = small.tile([_C, _B], f32)
    nc.vector.tensor_copy(out=ckT[:, :], in_=ckT_ps[:, :])

    mu = small.tile([_C, 1], f32)
    nc.vector.reduce_sum(out=mu[:, :], in_=cqT[:, :], axis=X)
    nc.scalar.mul(out=mu[:, :], in_=mu[:, :], mul=0.125)
    ckm = small.tile([_C, 1], f32)
    nc.vector.reduce_sum(out=ckm[:, :], in_=ckT[:, :], axis=X)
    nc.scalar.mul(out=ckm[:, :], in_=ckm[:, :], mul=0.125)
    dev = small.tile([_C, _B], f32)
    nc.vector.tensor_scalar(out=dev[:, :], in0=ckT[:, :], scalar1=ckm[0:_C, 0:1],
                            scalar2=None, op0=Alu.subtract)
    sq = small.tile([_C, _B], f32)
    nc.vector.tensor_mul(out=sq[:, :], in0=dev[:, :], in1=dev[:, :])
    var = small.tile([_C, 1], f32)
    nc.vector.reduce_sum(out=var[:, :], in_=sq[:, :], axis=X)
    nc.vector.tensor_scalar_mul(out=var[:, :], in0=var[:, :],
                                scalar1=float(1.0 / 7.0))
    std = small.tile([_C, 1], f32)
    nc.scalar.activation(out=std[:, :], in_=var[:, :], func=Act.Sqrt)
    sigma = small.tile([_C, 1], f32)
    nc.scalar.activation(out=sigma[:, :], in_=std[:, :], func=Act.Exp)
    nc.vector.tensor_scalar_add(out=sigma[:, :], in0=sigma[:, :], scalar1=1.0)
    nc.scalar.activation(out=sigma[:, :], in_=sigma[:, :], func=Act.Ln)
    logsig = small.tile([_C, 1], f32)
    nc.scalar.activation(out=logsig[:, :], in_=sigma[:, :], func=Act.Ln)
    rsig = small.tile([_C, 1], f32)
    nc.vector.reciprocal(out=rsig[:, :], in_=sigma[:, :])
    lsc = small.tile([_C, 1], f32)
    nc.vector.tensor_scalar_add(out=lsc[:, :], in0=logsig[:, :],
                                scalar1=float(0.5 * np.log(2 * np.pi)))
    tt = small.tile([_C, _B], f32)
    nc.vector.tensor_scalar(out=tt[:, :], in0=ckT[:, :], scalar1=mu[0:_C, 0:1],
                            scalar2=rsig[0:_C, 0:1],
                            op0=Alu.subtract, op1=Alu.mult)
    sq2 = small.tile([_C, _B], f32)
    nc.vector.tensor_mul(out=sq2[:, :], in0=tt[:, :], in1=tt[:, :])
    lp = small.tile([_C, _B], f32)
    nc.vector.tensor_scalar(out=lp[:, :], in0=sq2[:, :], scalar1=-0.5,
                            scalar2=lsc[0:_C, 0:1],
                            op0=Alu.mult, op1=Alu.subtract)
    lps = small.tile([_C, 1], f32)
    nc.vector.reduce_sum(out=lps[:, :], in_=lp[:, :], axis=X)
    lp_ps = ps.tile([1, 1], f32, tag="tiny2")
    nc.tensor.matmul(lp_ps[:, :], lps[:, :], one[0:_C, 0:1], start=True, stop=True)
    lp_sb = small.tile([1, 1], f32)
    nc.vector.tensor_copy(out=lp_sb[:, :], in_=lp_ps[:, :])

    # loss = -mean(log_prob) + ce = lp_sum*(-1/40) + ce_sum*(-1/8)
    l1 = small.tile([1, 1], f32)
    nc.vector.tensor_scalar_mul(out=l1[:, :], in0=lp_sb[:, :],
                                scalar1=float(-1.0 / 40.0))
    l2 = small.tile([1, 1], f32)
    nc.vector.tensor_scalar_mul(out=l2[:, :], in0=ce_sb[:, :],
                                scalar1=float(-1.0 / 8.0))
    lsum = small.tile([1, 1], f32)
    nc.vector.tensor_add(out=lsum[:, :], in0=l1[:, :], in1=l2[:, :])
    nc.sync.dma_start(out=loss_out[:], in_=lsum[:, :])

    # zflag = (cq[0,0] != max(cq[0])) as 0.0/1.0, replicated to 7 partitions
    m0 = small.tile([1, 1], f32)
    nc.vector.reduce_max(out=m0[:, :], in_=cq_sb[0:1, 0:_C], axis=X)
    eq = small.tile([1, 1], f32)
    nc.vector.tensor_tensor(out=eq[:, :], in0=cq_sb[0:1, 0:1], in1=m0[:, :],
                            op=Alu.is_equal)
    zf = small.tile([1, 1], f32)
    nc.vector.tensor_scalar(out=zf[:, :], in0=eq[:, :], scalar1=-1.0,
                            scalar2=1.0, op0=Alu.mult, op1=Alu.add)
    zf7_ps = ps.tile([7, 1], f32, tag="tiny")
    nc.tensor.matmul(zf7_ps[:, :], one[0:1, 0:7], zf[:, :], start=True, stop=True)
    zf7 = small.tile([7, 1], f32)
    nc.vector.tensor_copy(out=zf7[:, :], in_=zf7_ps[:, :])

    # ---- attn7: batch-0 attention for query rows 0..6 ----
    k0t_ps = ps.tile([_D, _H * _M], f32, tag="k0t")
    for hh in range(_H):
        nc.tensor.transpose(k0t_ps[:, hh * _M:(hh + 1) * _M],
                            ks0_sb[0:_M, hh * _D:(hh + 1) * _D],
                            eye[0:_M, 0:_M])
    k0t = small.tile([_D, _H * _M], f32)
    nc.vector.tensor_copy(out=k0t[:, :], in_=k0t_ps[:, :])

    s7_ps = ps.tile([7, _H * _M], f32, tag="s7")
    for hh in range(_H):
        nc.tensor.matmul(s7_ps[:, hh * _M:(hh + 1) * _M],
                         q7t[0:_D, hh * 7:(hh + 1) * 7],
                         k0t[0:_D, hh * _M:(hh + 1) * _M],
                         start=True, stop=True)
    s7 = small.tile([7, _H * _M], f32)
    nc.scalar.activation(out=s7[:, :], in_=s7_ps[:, :], func=Act.Copy,
                         scale=0.125)
    nc.vector.tensor_mul(out=s7[:, :], in0=s7[:, :], in1=tkeep[:, :])
    nc.vector.tensor_add(out=s7[:, :], in0=s7[:, :], in1=tneg[:, :])

    def seg_softmax(src, scale, tagp):
        """softmax over each 8-wide block of [7, 128]; exp((x-max)*scale)."""
        v3 = src[:, :].rearrange("p (h m) -> p h m", h=_H)
        mx = small.tile([7, _H], f32, tag=f"smx{tagp}")
        nc.vector.reduce_max(out=mx[:, :], in_=v3, axis=X)
        mxb = AP(tensor=mx.tensor, offset=mx.offset,
                 ap=list(mx[:, :].ap[:1]) + [[1, _H], [0, _M]])
        diff = small.tile([7, _H * _M], f32, tag=f"sdf{tagp}")
        nc.vector.tensor_tensor(out=diff[:, :], in0=src[:, :], in1=mxb,
                                op=Alu.subtract)
        ex = small.tile([7, _H * _M], f32, tag=f"sex{tagp}")
        nc.scalar.activation(out=ex[:, :], in_=diff[:, :], func=Act.Exp,
                             scale=scale)
        sm = small.tile([7, _H], f32, tag=f"ssm{tagp}")
        nc.vector.reduce_sum(out=sm[:, :],
                             in_=ex[:, :].rearrange("p (h m) -> p h m", h=_H),
                             axis=X)
        rs = small.tile([7, _H], f32, tag=f"srs{tagp}")
        nc.vector.reciprocal(out=rs[:, :], in_=sm[:, :])
        rsb = AP(tensor=rs.tensor, offset=rs.offset,
                 ap=list(rs[:, :].ap[:1]) + [[1, _H], [0, _M]])
        out = small.tile([7, _H * _M], f32, tag=f"sout{tagp}")
        nc.vector.tensor_tensor(out=out[:, :], in0=ex[:, :], in1=rsb,
                                op=Alu.mult)
        return out

    p7 = seg_softmax(s7, 1.0, "a")
    u7z = small.tile([7, _H * _M], f32)
    nc.vector.tensor_scalar_mul(out=u7z[:, :], in0=u7_s[:, :],
                                scalar1=zf7[0:7, 0:1])
    pm = small.tile([7, _H * _M], f32)
    nc.vector.tensor_max(out=pm[:, :], in0=p7[:, :], in1=u7z[:, :])
    attn7 = seg_softmax(pm, 8.0, "b")

    a7t_ps = ps.tile([_M, _H * 7], f32, tag="s7")
    for hh in range(_H):
        nc.tensor.transpose(a7t_ps[:, hh * 7:(hh + 1) * 7],
                            attn7[0:7, hh * _M:(hh + 1) * _M],
                            eye[0:7, 0:7])
    a7t = small.tile([_M, _H * 7], f32)
    nc.vector.tensor_copy(out=a7t[:, :], in_=a7t_ps[:, :])

    # ---- outputs that need Vs ----
    ru = small.tile([1, _H * _D], f32)
    for half in range(2):
        ru_ps = ps.tile([1, 512], f32, tag="ru")
        nc.tensor.matmul(ru_ps[:, :], one[0:_M, 0:1],
                         vs_sb[0:_M, half * 512:(half + 1) * 512],
                         start=True, stop=True)
        nc.scalar.activation(out=ru[:, half * 512:(half + 1) * 512],
                             in_=ru_ps[:, :], func=Act.Copy, scale=0.125)

    c7 = small.tile([7, _H * _D], f32)
    for half in range(2):
        c7_ps = ps.tile([7, 512], f32, tag="c7")
        for hx in range(8):
            hh = half * 8 + hx
            nc.tensor.matmul(c7_ps[:, hx * _D:(hx + 1) * _D],
                             a7t[0:_M, hh * 7:(hh + 1) * 7],
                             vs_sb[0:_M, hh * _D:(hh + 1) * _D],
                             start=True, stop=True)
        nc.vector.tensor_copy(out=c7[0:7, half * 512:(half + 1) * 512],
                              in_=c7_ps[:, :])

    # UBUF: [128, h*256]; partition p = rows q=4p..4p+3, all = row_u[h]
    ru4 = small.tile([1, _H * 256], f32)
    src_r4 = AP(tensor=ru[:, :].tensor, offset=ru[:, :].offset,
                ap=list(ru[:, :].ap[:1]) + [[_D, _H], [0, 4], [1, _D]])
    nc.vector.tensor_copy(out=ru4[:, :].rearrange("p (h r d) -> p h r d",
                                                  h=_H, r=4), in_=src_r4)
    ubuf = small.tile([128, _H * 256], f32)
    for hh in range(0, _H, 2):
        ub_ps = ps.tile([128, 512], f32, tag="big", bufs=2)
        nc.tensor.matmul(ub_ps[:, :], one[0:1, 0:128],
                         ru4[0:1, hh * 256:(hh + 2) * 256],
                         start=True, stop=True)
        nc.vector.tensor_copy(out=ubuf[:, hh * 256:(hh + 2) * 256],
                              in_=ub_ps[:, :])

    # ---- output DMAs ----
    # chunks 0..7: rows c*512 .. c*512+511, all uniform rows
    for c in range(8):
        dstc = AP(tensor=ctx_out.tensor, offset=ctx_out.offset + c * 512 * _D,
                  ap=[[4 * _D, 128], [_L * _D, _H], [_D, 4], [1, _D]])
        nc.sync.dma_start(out=dstc,
                          in_=ubuf[:, :].rearrange("p (h q d) -> p h q d",
                                                   h=_H, q=4))
    # overwrite rows q=0..6 of every head with the real attention rows
    # (must stay AFTER the chunk-0 DMA: same-tensor WAW ordering)
    dst7 = AP(tensor=ctx_out.tensor, offset=ctx_out.offset,
              ap=[[_D, 7], [_L * _D, _H], [1, _D]])
    nc.sync.dma_start(out=dst7,
                      in_=c7[0:7, :].rearrange("p (h d) -> p h d", h=_H))
    ctx.close()


def _prep_consts():
    """Host-side layout prep of weights/constants (no input arithmetic)."""
    pass


def kernel(**inputs):
    import os
    import sys
    if "jax" not in sys.modules and \
            os.environ.get("JAX_PLATFORMS", "").strip() == "cpu":
        # the NEFF executes through the axon PJRT backend; a cpu-only pin
        # (commonly set for reference computation) would hide the devices.
        os.environ["JAX_PLATFORMS"] = "axon,cpu"
    from concourse.bass_utils import run_bass_kernel_spmd

    Q = np.ascontiguousarray(inputs["Q"], dtype=np.float32)
    K = np.ascontiguousarray(inputs["K"], dtype=np.float32)
    V = np.ascontiguousarray(inputs["V"], dtype=np.float32)
    W_sk = np.asarray(inputs["W_sk"], dtype=np.float32)
    b_sk = np.asarray(inputs["b_sk"], dtype=np.float32)
    W_sv = np.asarray(inputs["W_sv"], dtype=np.float32)
    b_sv = np.asarray(inputs["b_sv"], dtype=np.float32)
    W_pc = np.asarray(inputs["W_pc"], dtype=np.float32)
    b_pc = np.asarray(inputs["b_pc"], dtype=np.float32)
    W_q = np.asarray(inputs["W_q"], dtype=np.float32)
    b_q = np.asarray(inputs["b_q"], dtype=np.float32)
    W_k = np.asarray(inputs["W_k"], dtype=np.float32)
    b_k = np.asarray(inputs["b_k"], dtype=np.float32)

    if "nc" not in _cache:
        _cache["nc"] = _build_nc()
    nc = _cache["nc"]

    # ---- host-side layout prep (pure reshape/transpose/replication) ----
    # WskT[p, lc*8+m] = W_sk[m, lc*128+p]
    # WskT[p, ((g*4+par)*8+m)] = W_sk[m, g*512 + 4*p + par]
    WskT = np.ascontiguousarray(
        W_sk.reshape(_M, 8, 128, 4).transpose(2, 1, 3, 0).reshape(128, 32 * _M))
    WsvT = np.ascontiguousarray(
        W_sv.reshape(_M, 8, 128, 4).transpose(2, 1, 3, 0).reshape(128, 32 * _M))
    # Wpcre[m, c*1024 + h*64 + d] = W_pc[c, h*512 + m*64 + d]
    Wpcre = np.ascontiguousarray(
        W_pc.reshape(_C, _H, _M, _D).transpose(2, 0, 1, 3).reshape(_M, _C * 1024))
    CONST = np.zeros((128, 779), np.float32)
    CONST[0:_M, 0] = b_sk
    CONST[0:_M, 1] = b_sv
    CONST[0:_B, 2:7] = b_pc[None, :]
    CONST[0:_B, 7:12] = b_q[None, :]
    CONST[0:_B, 12:17] = b_k[None, :]
    CONST[0:_C, 17:22] = W_q.T
    CONST[0:_C, 22:27] = W_k.T
    # Q7T[d, h*7+q] = Q[0, h, q, d]
    CONST[0:_D, 27:139] = Q[0, :, 0:7, :].transpose(2, 0, 1).reshape(_D, _H * 7)
    qs = np.arange(7)[:, None]
    ms = np.arange(_M)[None, :]
    keep = (ms > qs).astype(np.float32)                      # [7, 8]
    CONST[0:7, 139:267] = np.repeat(keep[:, None, :], _H, axis=1).reshape(7, -1)
    CONST[0:7, 267:395] = np.repeat(
        ((-1e9) * (1.0 - keep)).astype(np.float32)[:, None, :], _H,
        axis=1).reshape(7, -1)
    u = np.where(ms > qs, 1.0 / (7 - qs), 0.0).astype(np.float32)
    CONST[0:7, 395:523] = np.repeat(u[:, None, :], _H, axis=1).reshape(7, -1)
    CONST[0:128, 523:651] = np.eye(128, dtype=np.float32)
    CONST[0:128, 651:779] = 1.0

    common = dict(WskT=WskT, WsvT=WsvT, Wpcre=Wpcre, CONST=CONST)
    in_maps = [dict(Kb=np.ascontiguousarray(K[b]),
                    Vb=np.ascontiguousarray(V[b]), **common)
               for b in range(_NCORES)]

    last_err = None
    for attempt in range(3):
        try:
            res = run_bass_kernel_spmd(nc, in_maps,
                                       core_ids=list(range(_NCORES)))
            break
        except Exception as e:  # transient device wedge -> retry
            last_err = e
            import time
            time.sleep(15)
    else:
        raise last_err
    context = np.stack([res.results[b]["ctx"] for b in range(_NCORES)], axis=0)
    loss = np.float32(res.results[0]["loss"][0, 0])
    return context, loss
